# revision 26
# baseline (speedup 1.0000x reference)
"""Trainium2 kernel for BufferRetrievalHungarianMatcher.

Problem: outputs [16,256,2048] f32, targets [16,256,2048] f32.
  cost[b,n,o] = -<outputs[b,n,:], targets[b,o,:]>
  col[b] = Hungarian(cost[b]) (exact min-cost assignment, 256x256)
  return stack([arange(256), col], axis=1) -> [16,2,256] int32

Device side (8 NeuronCores, 2 batches/core): the memory-bound batched
matmul producing the cost slabs. Operands are pre-laid-out on the host so
the contraction dim (2048) lands on SBUF partitions (m-tile-major layout),
avoiding any on-chip transposes; the negation is folded into the host
layout pass. The exact per-sample Hungarian solve (tiny, sequential,
data-dependent) runs on the host on the device-computed cost slabs.
"""

import numpy as np

_NCORES = 8
_B, _N, _M = 16, 256, 2048
_BPC = _B // _NCORES      # batches per core
_MT = _M // 128           # 16 m-tiles of the contraction dim
_NT = _N // 128           # 2 n-tiles (PSUM partition tiles)
_CHUNK = 8                # m-tiles per input DMA chunk

LAST_RESULTS = None       # BassKernelResults of the most recent device run

_COMPUTE_DTYPE = "float32"  # "float32" | "float32r" (PE matmul operand dtype)
_RAW = False              # raw bacc builder vs Tile-framework builder
_nc_cache = {}


def _build_nc(compute_dtype: str = "float32"):
    """Build the SPMD Bass module (one NEFF, run on all 8 cores)."""
    import concourse.mybir as mybir
    from concourse import bacc
    from concourse.tile import TileContext

    f32 = getattr(mybir.dt, compute_dtype)
    nc = bacc.Bacc(
        trn_type="TRN2",
        target_bir_lowering=False,
        debug=False,
        num_devices=_NCORES,
    )
    # Host layout, per batch b and chunk ch (covering m-tiles 4ch..4ch+3):
    #   ab[b, ch, p, loc*256 + n]        = -outputs[2c+b, n, (4ch+loc)*128 + p]
    #   ab[b, ch, p, 1024 + loc*256 + o] =  targets[2c+b, o, (4ch+loc)*128 + p]
    # A and B chunks share one DRAM tensor/tile so each matmul depends on a
    # single input DMA (HW limits sync-wait slots per instruction).
    n_chunks = _MT // _CHUNK
    half = _CHUNK * 256
    ab = nc.dram_tensor(
        "ab", [_BPC, n_chunks, 128, 2 * half], f32, kind="ExternalInput"
    ).ap()
    # One output tensor per batch (separate tensors avoid false WAW deps
    # between the tail DMAs, which would exceed the 1-wait HWDGE limit):
    # cost{b}[p, nt*256 + o] = cost[2c+b, nt*128+p, o]
    of32 = mybir.dt.float32
    costs = [
        nc.dram_tensor(f"cost{b}", [128, _NT * 256], of32, kind="ExternalOutput").ap()
        for b in range(_BPC)
    ]

    # Per-batch DMA plan: (storage_chunk, first_m_tile_in_chunk, n_m_tiles).
    # The final pieces of the last batch are small so the PE tail after the
    # last DMA byte lands is short.
    plans = [[(c, 0, _CHUNK) for c in range(n_chunks)] for _ in range(_BPC)]
    plans[-1] = plans[-1][:-1] + [
        (n_chunks - 1, 0, 2),
        (n_chunks - 1, 2, 1),
        (n_chunks - 1, 3, 1),
    ]

    with TileContext(nc) as tc:
        with (
            tc.tile_pool(name="inp", bufs=1) as inp,
            tc.tile_pool(name="psum", bufs=2, space="PSUM") as psp,
            tc.tile_pool(name="outp", bufs=2) as outp,
        ):
            # Issue every input DMA up front on the SP (sync) HWDGE queue so
            # the input stream is never stalled behind an output DMA's wait
            # (the SP sequencer issues strictly in program order). Output
            # DMAs go on the Scalar-engine HWDGE queue instead.
            tiles_all = []
            for b in range(_BPC):
                tiles = []
                for i, (ch, loc0, k) in enumerate(plans[b]):
                    # [128, 2, k*256]: seg 0 = A m-tiles, seg 1 = B m-tiles
                    t = inp.tile(
                        [128, 2, k * 256], f32, tag=f"ab{b}_{i}", name=f"ab{b}_{i}"
                    )
                    src = ab[b, ch].rearrange("p (s x) -> p s x", s=2)[
                        :, :, loc0 * 256 : (loc0 + k) * 256
                    ]
                    nc.sync.dma_start(t, src)
                    tiles.append((t, loc0, k))
                tiles_all.append(tiles)

            for b in range(_BPC):
                psums = [
                    psp.tile([128, 256], of32, tag=f"c{nt}", name=f"c{nt}_{b}")
                    for nt in range(_NT)
                ]
                mt = 0
                for t, loc0, k in tiles_all[b]:
                    for i in range(k):
                        rhs = t[:, 1, i * 256 : (i + 1) * 256]
                        for nt in range(_NT):
                            lo = i * 256 + nt * 128
                            lhsT = t[:, 0, lo : lo + 128]
                            nc.tensor.matmul(
                                psums[nt],
                                lhsT,
                                rhs,
                                start=(mt == 0),
                                stop=(mt == _MT - 1),
                            )
                        mt += 1
                o_t = outp.tile([128, _NT * 256], of32, tag="o", name=f"o_{b}")
                # Two engines so the copies run in parallel at the tail.
                nc.scalar.copy(o_t[:, 0:256], psums[0])
                nc.vector.tensor_copy(o_t[:, 256:512], psums[1])
                nc.scalar.dma_start(costs[b], o_t)
    nc.compile()
    return nc


def _build_nc_raw(compute_dtype: str = "float32"):
    """Raw (non-Tile) variant: hand-placed semaphores, minimal preamble.

    Engine programs:
      SP:   all input DMA triggers immediately (one sem per piece), then a
            final wait for the two output DMAs.
      PE:   per piece, one wait on its DMA sem, then the matmul chains.
            The accumulation-chain stop matmuls signal pe_sem.
      ACT:  psum c0 -> out tile copy, then the output DMA (waits DVE copy).
      DVE:  psum c1 -> out tile copy, signals dve_sem.
    Each PSUM accumulator gets a private full bank (PE-write + copy-read of
    the same bank is a HW hazard).
    """
    import concourse.mybir as mybir
    from concourse import bacc
    from contextlib import ExitStack

    f32 = getattr(mybir.dt, compute_dtype)
    of32 = mybir.dt.float32
    nc = bacc.Bacc(
        trn_type="TRN2",
        target_bir_lowering=False,
        debug=False,
        num_devices=_NCORES,
    )
    n_chunks = _MT // _CHUNK
    half = _CHUNK * 256
    ab = nc.dram_tensor(
        "ab", [_BPC, n_chunks, 128, 2 * half], f32, kind="ExternalInput"
    ).ap()
    costs = [
        nc.dram_tensor(f"cost{b}", [128, _NT * 256], of32, kind="ExternalOutput").ap()
        for b in range(_BPC)
    ]

    plans = [[(c, 0, _CHUNK) for c in range(n_chunks)] for _ in range(_BPC)]
    plans[-1] = plans[-1][:-1] + [
        (n_chunks - 1, 0, 2),
        (n_chunks - 1, 2, 1),
        (n_chunks - 1, 3, 1),
    ]
    pieces = []  # (b, piece_idx_in_batch, ch, loc0, k)
    for b in range(_BPC):
        for i, (ch, loc0, k) in enumerate(plans[b]):
            pieces.append((b, i, ch, loc0, k))

    with ExitStack() as ctx:
        in_tiles = [
            nc.alloc_sbuf_tensor(f"ab{b}_{i}", [128, 2, k * 256], f32).ap()
            for (b, i, ch, loc0, k) in pieces
        ]
        out_tiles = [
            nc.alloc_sbuf_tensor(f"o_{b}", [128, _NT * 256], of32).ap()
            for b in range(_BPC)
        ]
        # one full PSUM bank per accumulator
        psums = [
            [
                nc.alloc_psum_tensor(f"c{nt}_{b}", [128, 512], of32).ap()[:, :256]
                for nt in range(_NT)
            ]
            for b in range(_BPC)
        ]
        in_sems = [
            ctx.enter_context(nc.semaphore(f"in_sem{j}")) for j in range(len(pieces))
        ]
        pe_sem = ctx.enter_context(nc.semaphore("pe_sem"))
        dve_sem = ctx.enter_context(nc.semaphore("dve_sem"))
        out_sem = ctx.enter_context(nc.semaphore("out_sem"))

        with nc.Block() as block:

            @block.sync
            def _(sync):
                for j, (b, i, ch, loc0, k) in enumerate(pieces):
                    src = ab[b, ch].rearrange("p (s x) -> p s x", s=2)[
                        :, :, loc0 * 256 : (loc0 + k) * 256
                    ]
                    sync.dma_start(in_tiles[j], src).then_inc(in_sems[j], 16)
                sync.wait_ge(out_sem, 16 * _BPC)

            @block.tensor
            def _(tensor):
                for j, (b, i, ch, loc0, k) in enumerate(pieces):
                    tensor.wait_ge(in_sems[j], 16)
                    t = in_tiles[j]
                    base_mt = ch * _CHUNK + loc0
                    for i2 in range(k):
                        mt = base_mt + i2
                        rhs = t[:, 1, i2 * 256 : (i2 + 1) * 256]
                        for nt in range(_NT):
                            lo = i2 * 256 + nt * 128
                            mm = tensor.matmul(
                                psums[b][nt],
                                t[:, 0, lo : lo + 128],
                                rhs,
                                start=(mt == 0),
                                stop=(mt == _MT - 1),
                            )
                            if mt == _MT - 1:
                                mm.then_inc(pe_sem, 1)

            @block.vector
            def _(vector):
                for b in range(_BPC):
                    vector.wait_ge(pe_sem, 2 * b + 2)
                    vector.tensor_copy(
                        out_tiles[b][:, 256:512], psums[b][1]
                    ).then_inc(dve_sem, 1)

            @block.scalar
            def _(scalar):
                for b in range(_BPC):
                    scalar.wait_ge(pe_sem, 2 * b + 1)
                    scalar.copy(out_tiles[b][:, 0:256], psums[b][0])
                    scalar.wait_ge(dve_sem, b + 1)
                    scalar.dma_start(costs[b], out_tiles[b]).then_inc(out_sem, 16)

    nc.compile()
    return nc


def _get_nc():
    key = (_COMPUTE_DTYPE, _RAW)
    if key not in _nc_cache:
        builder = _build_nc_raw if _RAW else _build_nc
        _nc_cache[key] = builder(_COMPUTE_DTYPE)
    return _nc_cache[key]


def _device_cost(outputs: np.ndarray, targets: np.ndarray) -> np.ndarray:
    """Compute cost[b,n,o] = -outputs[b]@targets[b].T on the 8 NeuronCores."""
    global LAST_RESULTS
    from concourse.bass_utils import run_bass_kernel_spmd

    # [B, N, M] -> m-tile-major transposed layout [B, n_chunks, 128, CHUNK*256]
    n_chunks = _MT // _CHUNK
    half = _CHUNK * 256

    def to_mtile_major(x):
        x = x.reshape(_B, _N, n_chunks, _CHUNK, 128).transpose(0, 2, 4, 3, 1)
        return np.ascontiguousarray(x, dtype=np.float32).reshape(
            _B, n_chunks, 128, half
        )

    ab = np.empty((_B, n_chunks, 128, 2 * half), dtype=np.float32)
    ab[..., :half] = to_mtile_major(outputs)
    np.negative(ab[..., :half], out=ab[..., :half])
    ab[..., half:] = to_mtile_major(targets)

    in_maps = [
        {"ab": ab[c * _BPC : (c + 1) * _BPC]} for c in range(_NCORES)
    ]
    res = run_bass_kernel_spmd(_get_nc(), in_maps, list(range(_NCORES)))
    LAST_RESULTS = res
    cost = np.empty((_B, _N, _N), dtype=np.float32)
    for c in range(_NCORES):
        for b in range(_BPC):
            out = res.results[c][f"cost{b}"]  # [128, NT*256]
            cost[c * _BPC + b] = (
                out.reshape(128, _NT, 256).transpose(1, 0, 2).reshape(_N, _N)
            )
    return cost


def _lap_numpy(cost: np.ndarray) -> np.ndarray:
    """Jonker-Volgenant shortest-augmenting-path LAP (e-maxx form), numpy.

    Fallback when scipy is unavailable. Matches
    scipy.optimize.linear_sum_assignment for square inputs.
    Returns col[row] int32 [n].
    """
    n = cost.shape[0]
    C = np.zeros((n + 1, n + 1), dtype=cost.dtype)
    C[1:, 1:] = cost
    INF = np.inf
    u = np.zeros(n + 1, cost.dtype)
    v = np.zeros(n + 1, cost.dtype)
    p = np.zeros(n + 1, np.int64)
    for i in range(1, n + 1):
        p[0] = i
        j0 = 0
        minv = np.full(n + 1, INF, cost.dtype)
        way = np.zeros(n + 1, np.int64)
        used = np.zeros(n + 1, bool)
        while True:
            used[j0] = True
            i0 = p[j0]
            cur = C[i0] - u[i0] - v
            better = (cur < minv) & ~used
            minv[better] = cur[better]
            way[better] = j0
            masked = np.where(used, INF, minv)
            j1 = int(np.argmin(masked))
            delta = masked[j1]
            add = np.where(used, delta, 0.0).astype(cost.dtype)
            np.add.at(u, p[used], delta)
            v[used] -= delta
            minv[~used] -= delta
            j0 = j1
            if p[j0] == 0:
                break
        while j0 != 0:
            j1 = way[j0]
            p[j0] = p[j1]
            j0 = j1
    col = np.zeros(n, np.int32)
    col[p[1:] - 1] = np.arange(n, dtype=np.int32)
    return col


def _solve_lap(cost: np.ndarray) -> np.ndarray:
    """Per-batch exact assignment: col indices [B, N] int32."""
    try:
        from scipy.optimize import linear_sum_assignment

        return np.stack(
            [
                linear_sum_assignment(cost[b])[1].astype(np.int32)
                for b in range(cost.shape[0])
            ]
        )
    except ImportError:
        return np.stack([_lap_numpy(cost[b]) for b in range(cost.shape[0])])


def kernel(outputs: np.ndarray, targets: np.ndarray) -> np.ndarray:
    outputs = np.asarray(outputs, dtype=np.float32)
    targets = np.asarray(targets, dtype=np.float32)
    cost = _device_cost(outputs, targets)
    col = _solve_lap(cost)
    rows = np.broadcast_to(np.arange(_N, dtype=np.int32), (_B, _N))
    return np.stack([rows, col], axis=1).astype(np.int32)
